# revision 27
# baseline (speedup 1.0000x reference)
"""Trainium2 Bass kernel for the entity-assignment loss.

Math: per sample b, C[i,j] = mean_d (yt[b,i,d]-yp[b,j,d])^2.
loss = mean_b ( min_perm sum_i C[i, perm(i)] / 8 ).

Since each permutation uses every row i and every column j exactly once,
  sum_i C[i, perm(i)] = (nt + np - 2 * sum_i dot(i, perm(i))) / 64
with nt, np per-sample constants, so only MAX over perms of the dot sum
is needed.  That max is computed meet-in-the-middle with two bitmask DPs
over 2^8 column-subset states: DP "A" assigns yp rows 0..3 (stage 0 is
a copy of G columns into singleton states, then 3 update stages), DP
"B" assigns yp rows 4..7.  Final: max_S A[S] + B[~S] via one dense add
with a reversed access pattern on B (invalid |S| != 4 states see a
-60000 sentinel on at least one side and never win the max).

Stage update: new[T] = max(new[T], old[T\i] + G[i, t]) for 8 hop
directions i; old/new alternate between two buffers (stale values are
maxes over valid increasing-stage sub-assignments and the combine only
reads popcount-4 states, so they are harmless).

Engine usage: DVE does the G multiplies/folds and all DP hops (the
narrow scalar_tensor_tensor hops are issue-bound at ~260ns; GpSimd
cannot run them - TensorScalarPtr is not a Pool opcode).  ScalarE does
the chunk-1 casts and the squared-norm totals; GpSimd does sentinel
memsets and half the input DMA descriptor generation so transfers start
~2us earlier.

Sharding: pure data parallelism, 256 samples per core across 8 cores;
the final mean is taken on the host.
"""

import os
import sys

if "/opt/trn_rl_repo" not in sys.path:
    sys.path.insert(0, "/opt/trn_rl_repo")

import numpy as np

B, N, D = 2048, 8, 64
N_CORES = 8
B_LOC = B // N_CORES        # 256 samples per core
NT = 2                      # two partition tiles of 128 samples
NEG = -60000.0              # f16-representable sentinel

TRACE = False
_CACHE = {}

CUSTOM_HOP = os.environ.get("K_CUSTOM", "1") == "1"
USE_QF = os.environ.get("K_QF", "1") == "1"


def _register_hop_op():
    """Custom DVE op out = max(in0 + s0, in1) — semantically the stock
    scalar_tensor_tensor(add, max) hop, but with a hand-written 2x_1P uop
    program (the stock TensorScalarPtr row only has a 1x program, so the
    stock hop runs at 1 elem/cycle even for packed f16).

    2x program: one 32-bit read per port carries two packed f16; blocks
    0-1 compute add+max for the lo halves, blocks 2-3 for the hi halves
    (lo result rides delay lane 5), write both halves per cycle.
    """
    import concourse.dve_ops as dops
    from concourse.dve_spec import Spec, Src0, Src1, C0, maxx, lower
    from concourse.dve_uop import (
        AluInp, DelayInp, DveOpSpec, InpSel, OutPath, OutSel, Trigger,
        UopConfig,
    )
    from concourse.dve_uop import AluOp as UAluOp

    for o in dops.OPS:
        if o.name == "HOP_MAX_ANT":
            return o

    hop_spec = Spec(
        body=maxx(Src0 + C0, Src1),
        reference=lambda in0, in1, s0, s1, imm2: np.maximum(
            in0.astype(np.float32) + s0, in1).astype(np.float32),
    )
    op = dops.DveOp("HOP_MAX_ANT", hop_spec, subdim=False, uops_sha={})
    dops.OPS.append(op)
    row = dops._CUSTOM_DVE_ROW_BASE + len(dops.OPS) - 1
    dops._SUB_OPCODE_FOR_NAME["HOP_MAX_ANT"] = row
    dops.CUSTOM_DVE_SPECS["HOP_MAX_ANT"] = hop_spec

    uops_1x = lower(hop_spec, ver="v3")

    u = UopConfig()
    u.require_inp0 = 1
    u.require_inp1 = 1
    u.trigger = (Trigger.SRC_TENSOR_DONE, Trigger.NONE, Trigger.NONE)
    # lane k feeds block 0's delay chain k-1
    for src, lane in ((InpSel.SRC_0, 1), (InpSel.CONST_0, 2), (InpSel.SRC_1, 3),
                      (InpSel.SRC_0_HI, 4), (InpSel.SRC_1_HI, 5)):
        u.enable_input(src, lane)
    dp = u.datapath_config
    dp[0].enable_alu(UAluOp.ADD, AluInp.PREV_DELAY_0, AluInp.PREV_DELAY_1)
    dp[0].pass_through_delay(1, 2, 3, 4)
    dp[1].enable_alu(UAluOp.MAX, AluInp.PREV_ALU_OUT, AluInp.PREV_DELAY_2)
    dp[1].pass_through_delay(1, 3, 4)
    dp[2].enable_alu(UAluOp.ADD, AluInp.PREV_DELAY_3, AluInp.PREV_DELAY_1)
    dp[2].enable_delay_from_src(DelayInp.PREV_ALU_OUT, 5)   # park lo result
    dp[2].pass_through_delay(4)
    dp[3].enable_alu(UAluOp.MAX, AluInp.PREV_ALU_OUT, AluInp.PREV_DELAY_4)
    dp[3].pass_through_delay(5)
    for b in range(4, 8):
        dp[b].pass_through_alu()
        dp[b].pass_through_delay(5)
    u.enable_output(OutSel.DELAY_5, OutPath.WR0_LO)
    u.enable_output(OutSel.ALU_OUT, OutPath.WR0_HI)

    dspec = DveOpSpec(name="HOP_MAX_ANT", opcode=row, uops=uops_1x,
                      uops_2x=[u], perf_max=1, rd1_en=True)
    dspec.validate("v3")
    dops._COMPILE_CACHE[("HOP_MAX_ANT", "v3")] = dspec
    return op


def _register_combine_op():
    """Custom DVE op out = in0 + in1, accum_out = max(out): fuses the
    meet-in-the-middle combine A[S] + B[~S] with the max-reduction."""
    import concourse.dve_ops as dops
    from concourse.dve_spec import Spec, Src0, Src1, lower
    from concourse.dve_uop import AluOp as UAluOp

    for o in dops.OPS:
        if o.name == "COMBINE_MAX_ANT":
            return o

    def _ref(in0, in1, s0, s1, imm2):
        b = (in0.astype(np.float32) + in1).astype(np.float32)
        return b, b.reshape(b.shape[0], -1).max(axis=-1, keepdims=True)

    spec = Spec(body=Src0 + Src1, accum=UAluOp.MAX, reference=_ref)
    op = dops.DveOp("COMBINE_MAX_ANT", spec, subdim=False, uops_sha={})
    dops.OPS.append(op)
    row = dops._CUSTOM_DVE_ROW_BASE + len(dops.OPS) - 1
    dops._SUB_OPCODE_FOR_NAME["COMBINE_MAX_ANT"] = row
    dops.CUSTOM_DVE_SPECS["COMBINE_MAX_ANT"] = spec
    from concourse.dve_uop import DveOpSpec
    dspec = DveOpSpec(name="COMBINE_MAX_ANT", opcode=row,
                      uops=lower(spec, ver="v3"), rd1_en=True)
    dspec.validate("v3")
    dops._COMPILE_CACHE[("COMBINE_MAX_ANT", "v3")] = dspec
    return op


def _register_quadfold_op():
    """Custom DVE op out[k] = in0[2k] + in0[2k+1] + in1[2k] + in1[2k+1].

    Runs ONLY in 2x_1P mode: each port's 32-bit read carries two packed
    f16, so one pass folds 4 source elements into 1 result per cycle --
    two binary-fold levels of the dot-product reduction in a single
    instruction at the same input rate as a stock tensor_tensor add.
    Call sites must guarantee 2x eligibility (f16, stride-1 pairs,
    4-byte-aligned bases); the 1x table slot holds the same program and
    would produce garbage if the engine ever fell back.
    """
    import concourse.dve_ops as dops
    from concourse.dve_spec import Spec, Src0, Src1
    from concourse.dve_uop import (
        AluInp, DelayInp, DveOpSpec, InpSel, OutPath, OutSel, Trigger,
        UopConfig,
    )
    from concourse.dve_uop import AluOp as UAluOp

    for o in dops.OPS:
        if o.name == "QUADFOLD_ANT":
            return o

    def _ref(in0, in1, s0, s1, imm2):
        a = in0.astype(np.float32).reshape(in0.shape[0], -1, 2).sum(axis=-1)
        b = in1.astype(np.float32).reshape(in1.shape[0], -1, 2).sum(axis=-1)
        return (a + b).astype(np.float32)

    # body is only documentation / CoreSim semantics; HW runs the
    # hand-written program below
    spec = Spec(body=Src0 + Src1, reference=_ref)
    op = dops.DveOp("QUADFOLD_ANT", spec, subdim=False, uops_sha={})
    dops.OPS.append(op)
    row = dops._CUSTOM_DVE_ROW_BASE + len(dops.OPS) - 1
    dops._SUB_OPCODE_FOR_NAME["QUADFOLD_ANT"] = row
    dops.CUSTOM_DVE_SPECS["QUADFOLD_ANT"] = spec

    u = UopConfig()
    u.require_inp0 = 1
    u.require_inp1 = 1
    u.trigger = (Trigger.SRC_TENSOR_DONE, Trigger.NONE, Trigger.NONE)
    for src, lane in ((InpSel.SRC_0, 1), (InpSel.SRC_0_HI, 2),
                      (InpSel.SRC_1, 3), (InpSel.SRC_1_HI, 4)):
        u.enable_input(src, lane)
    dp = u.datapath_config
    dp[0].enable_alu(UAluOp.ADD, AluInp.PREV_DELAY_0, AluInp.PREV_DELAY_1)
    dp[0].pass_through_delay(2, 3)
    dp[1].enable_alu(UAluOp.ADD, AluInp.PREV_DELAY_2, AluInp.PREV_DELAY_3)
    dp[1].enable_delay_from_src(DelayInp.PREV_ALU_OUT, 4)   # park lo sum
    dp[2].enable_alu(UAluOp.ADD, AluInp.PREV_ALU_OUT, AluInp.PREV_DELAY_4)
    for b in range(3, 8):
        dp[b].pass_through_alu()
    u.enable_output(OutSel.ALU_OUT, OutPath.WR0_LO)

    dspec = DveOpSpec(name="QUADFOLD_ANT", opcode=row, uops=[u],
                      uops_2x=[u], perf_max=1, rd1_en=True)
    dspec.validate("v3")
    dops._COMPILE_CACHE[("QUADFOLD_ANT", "v3")] = dspec
    return op


def _build():
    import concourse.bacc as bacc
    import concourse.mybir as mybir
    from concourse.tile import TileContext

    f32 = mybir.dt.float32
    f16 = mybir.dt.float16
    Alu = mybir.AluOpType
    Act = mybir.ActivationFunctionType

    nc = bacc.Bacc("TRN2", target_bir_lowering=False, debug=False)
    yt_d = nc.declare_dram_parameter("yt", [B_LOC, N * D], f16, isOutput=False)
    yp_d = nc.declare_dram_parameter("yp", [B_LOC, N * D], f16, isOutput=False)
    out_d = nc.declare_dram_parameter("out", [128, NT], f32, isOutput=True)

    with TileContext(nc) as tc:
        with (
            tc.tile_pool(name="io", bufs=1) as io_pool,
            tc.tile_pool(name="res", bufs=1) as res_pool,
        ):
            # ---- persistent tiles ----
            G32 = res_pool.tile([128, NT * N * N], f32, tag="G32")
            # DP buffers, 4 units: [A-c0 | A-c1 | B-c0 | B-c1];
            # Mb holds even stages (0, 2), Pb odd stages (1, 3).
            Mb = res_pool.tile([128, 4 * 256], f16, tag="Mb")
            Pb = res_pool.tile([128, 4 * 256], f16, tag="Pb")
            Mc = res_pool.tile([128, NT * 256], f16, tag="Mc")
            dmax = res_pool.tile([128, NT], f16, tag="dmax")
            ntt = res_pool.tile([128, NT], f32, tag="ntt")
            npt = res_pool.tile([128, NT], f32, tag="npt")
            s_all = res_pool.tile([128, NT], f32, tag="s_all")
            loss_t = res_pool.tile([128, NT], f32, tag="loss")
            sq = res_pool.tile([128, N * D], f32, tag="sq")
            warm = res_pool.tile([128, 1], f32, tag="warm")

            # DP sentinel init on GpSimd (no dependencies; runs at t=0)
            nc.gpsimd.memset(Mb[:, :], NEG)
            nc.gpsimd.memset(Pb[:, :], NEG)
            # preload the ScalarE activation table before any data arrives
            nc.scalar.activation(out=warm[:, :], in_=warm[:, :], func=Act.Square)

            # ---- loads (inputs are pre-cast to f16 on the host, halving
            # DMA bytes); descriptor generation split across two queues ----
            yt_h = [io_pool.tile([128, N * D], f16, tag=f"yth{c}", name=f"yth{c}")
                    for c in range(NT)]
            yp_h = [io_pool.tile([128, N * D], f16, tag=f"yph{c}", name=f"yph{c}")
                    for c in range(NT)]
            nc.sync.dma_start(out=yt_h[0][:, :], in_=yt_d[0:128, :])
            nc.scalar.dma_start(out=yp_h[0][:, :], in_=yp_d[0:128, :])
            nc.sync.dma_start(out=yt_h[1][:, :], in_=yt_d[128:256, :])
            nc.scalar.dma_start(out=yp_h[1][:, :], in_=yp_d[128:256, :])

            # per-sample squared-norm totals on the (otherwise idle) ScalarE
            for c in range(NT):
                nc.scalar.activation(out=sq[:, :], in_=yt_h[c][:, :],
                                     func=Act.Square, accum_out=ntt[:, c:c + 1])
                nc.scalar.activation(out=sq[:, :], in_=yp_h[c][:, :],
                                     func=Act.Square, accum_out=npt[:, c:c + 1])

            # ---- G[c][i*8+j] = dot(yt_i, yp_j): per-chunk broadcast
            # multiply, then one shared fold tree over both chunks ----
            prod = res_pool.tile([128, NT * N * N * D], f16, tag="prod")
            Q = NT * N * N          # 128 dot-product segments
            pv = prod.rearrange("p (q d) -> p q d", d=D)
            half = res_pool.tile([128, Q * D // 2], f16, tag="half")
            hv = half.rearrange("p (q d) -> p q d", d=D // 2)
            for c in range(NT):
                # fold1 of chunk c-1 is emitted between the two multiplies
                # so the DVE has work while chunk-1 finishes its DMA
                yt_b = yt_h[c].rearrange("p (i d) -> p i d", d=D).unsqueeze(2) \
                    .broadcast_to([128, N, N, D])
                yp_b = yp_h[c].rearrange("p (j d) -> p j d", d=D).unsqueeze(1) \
                    .broadcast_to([128, N, N, D])
                pc = prod[:, c * N * N * D:(c + 1) * N * N * D]
                nc.vector.tensor_tensor(
                    out=pc.rearrange("p (i j d) -> p i j d", j=N, d=D),
                    in0=yt_b, in1=yp_b, op=Alu.mult)
                qlo, qhi = c * Q // 2, (c + 1) * Q // 2
                nc.vector.tensor_tensor(
                    out=hv[:, qlo:qhi, :], in0=pv[:, qlo:qhi, 0:D // 2],
                    in1=pv[:, qlo:qhi, D // 2:D], op=Alu.add)
            qf_op = _register_quadfold_op() if (CUSTOM_HOP and USE_QF) else None
            eighth = res_pool.tile([128, Q * D // 8], f16, tag="eighth")
            if qf_op is not None:
                # quad-folds: d32 -> d8 -> d2, then a stock segmented
                # reduce d2 -> 1 that also converts to f32
                h4 = half.rearrange("p (k e) -> p k e", e=2)
                bi = nc.vector._custom_dve(
                    qf_op, out=eighth[:, :],
                    in0=h4[:, 0::2, :], in1=h4[:, 1::2, :])
                bi.ins.perf_max = 1
                half2 = res_pool.tile([128, Q * D // 32], f16, tag="half2")
                e4 = eighth.rearrange("p (k e) -> p k e", e=2)
                bi = nc.vector._custom_dve(
                    qf_op, out=half2[:, :],
                    in0=e4[:, 0::2, :], in1=e4[:, 1::2, :])
                bi.ins.perf_max = 1
                nc.vector.tensor_reduce(
                    out=G32[:, :],
                    in_=half2.rearrange("p (q d) -> p q d", d=2),
                    axis=mybir.AxisListType.X, op=Alu.add)
            else:
                quart = res_pool.tile([128, Q * D // 4], f16, tag="quart")
                qv = quart.rearrange("p (q d) -> p q d", d=D // 4)
                nc.vector.tensor_tensor(
                    out=qv, in0=hv[:, :, 0:D // 4], in1=hv[:, :, D // 4:D // 2],
                    op=Alu.add)
                ev = eighth.rearrange("p (q d) -> p q d", d=D // 8)
                nc.vector.tensor_tensor(
                    out=ev, in0=qv[:, :, 0:D // 8], in1=qv[:, :, D // 8:D // 4],
                    op=Alu.add)
                nc.vector.tensor_reduce(
                    out=G32[:, :], in_=ev, axis=mybir.AxisListType.X, op=Alu.add)

            # ---- stage 0: singleton states M[{2^i}] = G[i, t0] with
            # t0 = 0 (side A) / 4 (side B); one copy per i-pair covers all
            # 4 units: dst dims (side: +512, chunk: +256, pair: ci) from
            # src dims (t0: +4, chunk: +64, row: +8). ----
            # (on ScalarE: it is idle and this keeps the DVE on the DP)
            g4 = G32.rearrange("p (c i j) -> p c i j", c=NT, i=N)
            m4 = Mb.rearrange("p (u c s) -> p u c s", u=2, c=NT)
            for i in range(0, N, 2):
                ci = 2 ** i
                dst = m4[:, :, :, ci:2 * ci + 1:ci]
                src = g4[:, :, i:i + 2, 0:5:4].rearrange("p c i j -> p j c i")
                nc.scalar.copy(dst, src)

            # ---- DP stages t = 1..3 ----
            # hop (side, c, i, t): scalar column c*64 + i*8 + t + 4*side
            hop_op = _register_hop_op() if CUSTOM_HOP else None
            bufs = [Mb, Pb]
            for t in range(1, 4):
                old, new = bufs[(t + 1) % 2], bufs[t % 2]
                for i in range(N):
                    ci = 2 ** i
                    for u in range(4):
                        side, c = u >> 1, u & 1
                        base = u * 256
                        col = c * 64 + i * 8 + t + 4 * side
                        ov = old[:, base:base + 256].rearrange(
                            "p (a b s) -> p a b s", b=2, s=ci)
                        nv = new[:, base:base + 256].rearrange(
                            "p (a b s) -> p a b s", b=2, s=ci)
                        if hop_op is not None:
                            bi = nc.vector._custom_dve(
                                hop_op, out=nv[:, :, 1, :], in0=ov[:, :, 0, :],
                                in1=nv[:, :, 1, :],
                                s0=G32[:, col:col + 1])
                            bi.ins.perf_max = 1
                        else:
                            nc.vector.scalar_tensor_tensor(
                                out=nv[:, :, 1, :], in0=ov[:, :, 0, :],
                                scalar=G16[:, col:col + 1],
                                in1=nv[:, :, 1, :], op0=Alu.add, op1=Alu.max)

            # ---- combine: dmax[c] = max_S A_c[S] + B_c[~S] (fused custom
            # op: add with a MAX accumulator) ----
            comb_op = _register_combine_op()
            fin = bufs[1]                                  # after stage 3
            for c in range(NT):
                a_ap = fin[:, c * 256:(c + 1) * 256]
                b_rev = fin[:, 512 + c * 256:512 + (c + 1) * 256][:, 255::-1]
                nc.vector._custom_dve(
                    comb_op, out=Mc[:, c * 256:(c + 1) * 256],
                    in0=a_ap, in1=b_rev,
                    accum_out=dmax[:, c:c + 1])

            # loss per sample: nt + np - 2*dmax  (final /(64*8*B) on host)
            nc.vector.tensor_add(s_all[:, :], ntt[:, :], npt[:, :])
            nc.vector.scalar_tensor_tensor(
                out=loss_t[:, :], in0=dmax[:, :], scalar=-2.0,
                in1=s_all[:, :], op0=Alu.mult, op1=Alu.add)
            nc.sync.dma_start(out=out_d[:, :], in_=loss_t[:, :])
    nc.compile()
    return nc


def kernel(y_true: np.ndarray, y_pred: np.ndarray) -> np.ndarray:
    from concourse.bass_utils import run_bass_kernel_spmd

    if "nc" not in _CACHE:
        _CACHE["nc"] = _build()
    nc = _CACHE["nc"]

    yt = np.ascontiguousarray(
        np.asarray(y_true, dtype=np.float32).astype(np.float16)).reshape(B, N * D)
    yp = np.ascontiguousarray(
        np.asarray(y_pred, dtype=np.float32).astype(np.float16)).reshape(B, N * D)

    in_maps = [
        {
            "yt": np.ascontiguousarray(yt[c * B_LOC:(c + 1) * B_LOC]),
            "yp": np.ascontiguousarray(yp[c * B_LOC:(c + 1) * B_LOC]),
        }
        for c in range(N_CORES)
    ]
    res = run_bass_kernel_spmd(nc, in_maps, list(range(N_CORES)), trace=TRACE)
    _CACHE["last_results"] = res
    vals = np.concatenate([np.asarray(r["out"], dtype=np.float64).reshape(-1)
                           for r in res.results])
    loss = vals.mean() / (D * N)
    return np.float32(loss)


# revision 30
# speedup vs baseline: 1.1776x; 1.1776x over previous
"""Trainium2 Bass kernel for the entity-assignment loss.

Math: per sample b, C[i,j] = mean_d (yt[b,i,d]-yp[b,j,d])^2.
loss = mean_b ( min_perm sum_i C[i, perm(i)] / 8 ).

Since each permutation uses every row i and every column j exactly once,
  sum_i C[i, perm(i)] = (nt + np - 2 * sum_i dot(i, perm(i))) / 64
with nt, np per-sample constants, so only MAX over perms of the dot sum
is needed.  That max is computed meet-in-the-middle with two bitmask DPs
over 2^8 column-subset states: DP "A" assigns yp rows 0..3 (stage 0 is
a copy of G columns into singleton states, then 3 update stages), DP
"B" assigns yp rows 4..7.  Final: max_S A[S] + B[~S] via one dense add
with a reversed access pattern on B (invalid |S| != 4 states see a
-60000 sentinel on at least one side and never win the max).

Stage update: new[T] = max(new[T], old[T\i] + G[i, t]) for 8 hop
directions i; old/new alternate between two buffers (stale values are
maxes over valid increasing-stage sub-assignments and the combine only
reads popcount-4 states, so they are harmless).

Engine usage: DVE does the G multiplies/folds and all DP hops (the
narrow scalar_tensor_tensor hops are issue-bound at ~260ns; GpSimd
cannot run them - TensorScalarPtr is not a Pool opcode).  ScalarE does
the chunk-1 casts and the squared-norm totals; GpSimd does sentinel
memsets and half the input DMA descriptor generation so transfers start
~2us earlier.

Sharding: pure data parallelism, 256 samples per core across 8 cores;
the final mean is taken on the host.
"""

import os
import sys

if "/opt/trn_rl_repo" not in sys.path:
    sys.path.insert(0, "/opt/trn_rl_repo")

import numpy as np

B, N, D = 2048, 8, 64
N_CORES = 8
B_LOC = B // N_CORES        # 256 samples per core
NT = 2                      # two partition tiles of 128 samples
NEG = -60000.0              # f16-representable sentinel

TRACE = False
_CACHE = {}

CUSTOM_HOP = os.environ.get("K_CUSTOM", "1") == "1"
USE_QF = os.environ.get("K_QF", "1") == "1"


def _register_hop_op():
    """Custom DVE op out = max(in0 + s0, in1) — semantically the stock
    scalar_tensor_tensor(add, max) hop, but with a hand-written 2x_1P uop
    program (the stock TensorScalarPtr row only has a 1x program, so the
    stock hop runs at 1 elem/cycle even for packed f16).

    2x program: one 32-bit read per port carries two packed f16; blocks
    0-1 compute add+max for the lo halves, blocks 2-3 for the hi halves
    (lo result rides delay lane 5), write both halves per cycle.
    """
    import concourse.dve_ops as dops
    from concourse.dve_spec import Spec, Src0, Src1, C0, maxx, lower
    from concourse.dve_uop import (
        AluInp, DelayInp, DveOpSpec, InpSel, OutPath, OutSel, Trigger,
        UopConfig,
    )
    from concourse.dve_uop import AluOp as UAluOp

    for o in dops.OPS:
        if o.name == "HOP_MAX_ANT":
            return o

    hop_spec = Spec(
        body=maxx(Src0 + C0, Src1),
        reference=lambda in0, in1, s0, s1, imm2: np.maximum(
            in0.astype(np.float32) + s0, in1).astype(np.float32),
    )
    op = dops.DveOp("HOP_MAX_ANT", hop_spec, subdim=False, uops_sha={})
    dops.OPS.append(op)
    row = dops._CUSTOM_DVE_ROW_BASE + len(dops.OPS) - 1
    dops._SUB_OPCODE_FOR_NAME["HOP_MAX_ANT"] = row
    dops.CUSTOM_DVE_SPECS["HOP_MAX_ANT"] = hop_spec

    uops_1x = lower(hop_spec, ver="v3")

    u = UopConfig()
    u.require_inp0 = 1
    u.require_inp1 = 1
    u.trigger = (Trigger.SRC_TENSOR_DONE, Trigger.NONE, Trigger.NONE)
    # lane k feeds block 0's delay chain k-1
    for src, lane in ((InpSel.SRC_0, 1), (InpSel.CONST_0, 2), (InpSel.SRC_1, 3),
                      (InpSel.SRC_0_HI, 4), (InpSel.SRC_1_HI, 5)):
        u.enable_input(src, lane)
    dp = u.datapath_config
    dp[0].enable_alu(UAluOp.ADD, AluInp.PREV_DELAY_0, AluInp.PREV_DELAY_1)
    dp[0].pass_through_delay(1, 2, 3, 4)
    dp[1].enable_alu(UAluOp.MAX, AluInp.PREV_ALU_OUT, AluInp.PREV_DELAY_2)
    dp[1].pass_through_delay(1, 3, 4)
    dp[2].enable_alu(UAluOp.ADD, AluInp.PREV_DELAY_3, AluInp.PREV_DELAY_1)
    dp[2].enable_delay_from_src(DelayInp.PREV_ALU_OUT, 5)   # park lo result
    dp[2].pass_through_delay(4)
    dp[3].enable_alu(UAluOp.MAX, AluInp.PREV_ALU_OUT, AluInp.PREV_DELAY_4)
    dp[3].pass_through_delay(5)
    for b in range(4, 8):
        dp[b].pass_through_alu()
        dp[b].pass_through_delay(5)
    u.enable_output(OutSel.DELAY_5, OutPath.WR0_LO)
    u.enable_output(OutSel.ALU_OUT, OutPath.WR0_HI)

    dspec = DveOpSpec(name="HOP_MAX_ANT", opcode=row, uops=uops_1x,
                      uops_2x=[u], perf_max=1, rd1_en=True)
    dspec.validate("v3")
    dops._COMPILE_CACHE[("HOP_MAX_ANT", "v3")] = dspec
    return op


def _register_combine_op():
    """Custom DVE op out = in0 + in1, accum_out = max(out): fuses the
    meet-in-the-middle combine A[S] + B[~S] with the max-reduction."""
    import concourse.dve_ops as dops
    from concourse.dve_spec import Spec, Src0, Src1, lower
    from concourse.dve_uop import AluOp as UAluOp

    for o in dops.OPS:
        if o.name == "COMBINE_MAX_ANT":
            return o

    def _ref(in0, in1, s0, s1, imm2):
        b = (in0.astype(np.float32) + in1).astype(np.float32)
        return b, b.reshape(b.shape[0], -1).max(axis=-1, keepdims=True)

    spec = Spec(body=Src0 + Src1, accum=UAluOp.MAX, reference=_ref)
    op = dops.DveOp("COMBINE_MAX_ANT", spec, subdim=False, uops_sha={})
    dops.OPS.append(op)
    row = dops._CUSTOM_DVE_ROW_BASE + len(dops.OPS) - 1
    dops._SUB_OPCODE_FOR_NAME["COMBINE_MAX_ANT"] = row
    dops.CUSTOM_DVE_SPECS["COMBINE_MAX_ANT"] = spec
    from concourse.dve_uop import DveOpSpec
    dspec = DveOpSpec(name="COMBINE_MAX_ANT", opcode=row,
                      uops=lower(spec, ver="v3"), rd1_en=True)
    dspec.validate("v3")
    dops._COMPILE_CACHE[("COMBINE_MAX_ANT", "v3")] = dspec
    return op


def _register_quadfold_op():
    """Custom DVE op out[k] = in0[2k] + in0[2k+1] + in1[2k] + in1[2k+1].

    Runs ONLY in 2x_1P mode: each port's 32-bit read carries two packed
    f16, so one pass folds 4 source elements into 1 result per cycle --
    two binary-fold levels of the dot-product reduction in a single
    instruction at the same input rate as a stock tensor_tensor add.
    Call sites must guarantee 2x eligibility (f16, stride-1 pairs,
    4-byte-aligned bases); the 1x table slot holds the same program and
    would produce garbage if the engine ever fell back.
    """
    import concourse.dve_ops as dops
    from concourse.dve_spec import Spec, Src0, Src1
    from concourse.dve_uop import (
        AluInp, DelayInp, DveOpSpec, InpSel, OutPath, OutSel, Trigger,
        UopConfig,
    )
    from concourse.dve_uop import AluOp as UAluOp

    for o in dops.OPS:
        if o.name == "QUADFOLD_ANT":
            return o

    def _ref(in0, in1, s0, s1, imm2):
        a = in0.astype(np.float32).reshape(in0.shape[0], -1, 2).sum(axis=-1)
        b = in1.astype(np.float32).reshape(in1.shape[0], -1, 2).sum(axis=-1)
        return (a + b).astype(np.float32)

    # body is only documentation / CoreSim semantics; HW runs the
    # hand-written program below
    spec = Spec(body=Src0 + Src1, reference=_ref)
    op = dops.DveOp("QUADFOLD_ANT", spec, subdim=False, uops_sha={})
    dops.OPS.append(op)
    row = dops._CUSTOM_DVE_ROW_BASE + len(dops.OPS) - 1
    dops._SUB_OPCODE_FOR_NAME["QUADFOLD_ANT"] = row
    dops.CUSTOM_DVE_SPECS["QUADFOLD_ANT"] = spec

    u = UopConfig()
    u.require_inp0 = 1
    u.require_inp1 = 1
    u.trigger = (Trigger.SRC_TENSOR_DONE, Trigger.NONE, Trigger.NONE)
    for src, lane in ((InpSel.SRC_0, 1), (InpSel.SRC_0_HI, 2),
                      (InpSel.SRC_1, 3), (InpSel.SRC_1_HI, 4)):
        u.enable_input(src, lane)
    dp = u.datapath_config
    dp[0].enable_alu(UAluOp.ADD, AluInp.PREV_DELAY_0, AluInp.PREV_DELAY_1)
    dp[0].pass_through_delay(2, 3)
    dp[1].enable_alu(UAluOp.ADD, AluInp.PREV_DELAY_2, AluInp.PREV_DELAY_3)
    dp[1].enable_delay_from_src(DelayInp.PREV_ALU_OUT, 4)   # park lo sum
    dp[2].enable_alu(UAluOp.ADD, AluInp.PREV_ALU_OUT, AluInp.PREV_DELAY_4)
    for b in range(3, 8):
        dp[b].pass_through_alu()
    u.enable_output(OutSel.ALU_OUT, OutPath.WR0_LO)

    dspec = DveOpSpec(name="QUADFOLD_ANT", opcode=row, uops=[u],
                      uops_2x=[u], perf_max=1, rd1_en=True)
    dspec.validate("v3")
    dops._COMPILE_CACHE[("QUADFOLD_ANT", "v3")] = dspec
    return op


def _build():
    import concourse.bacc as bacc
    import concourse.mybir as mybir
    from concourse.tile import TileContext

    f32 = mybir.dt.float32
    f16 = mybir.dt.float16
    Alu = mybir.AluOpType
    Act = mybir.ActivationFunctionType

    nc = bacc.Bacc("TRN2", target_bir_lowering=False, debug=False)
    yt_d = nc.declare_dram_parameter("yt", [B_LOC, N * D], f16, isOutput=False)
    yp_d = nc.declare_dram_parameter("yp", [B_LOC, N * D], f16, isOutput=False)
    out_d = nc.declare_dram_parameter("out", [128, NT], f32, isOutput=True)

    with TileContext(nc) as tc:
        with (
            tc.tile_pool(name="io", bufs=1) as io_pool,
            tc.tile_pool(name="res", bufs=1) as res_pool,
        ):
            # ---- persistent tiles ----
            G32 = res_pool.tile([128, NT * N * N], f32, tag="G32")
            # DP buffers, 4 units: [A-c0 | A-c1 | B-c0 | B-c1];
            # Mb holds even stages (0, 2), Pb odd stages (1, 3).
            Mb = res_pool.tile([128, 4 * 256], f16, tag="Mb")
            Pb = res_pool.tile([128, 4 * 256], f16, tag="Pb")
            Mc = res_pool.tile([128, NT * 256], f16, tag="Mc")
            dmax = res_pool.tile([128, NT], f16, tag="dmax")
            ntt = res_pool.tile([128, NT], f32, tag="ntt")
            npt = res_pool.tile([128, NT], f32, tag="npt")
            s_all = res_pool.tile([128, NT], f32, tag="s_all")
            loss_t = res_pool.tile([128, NT], f32, tag="loss")
            sq = res_pool.tile([128, N * D], f32, tag="sq")
            warm = res_pool.tile([128, 1], f32, tag="warm")

            # DP sentinel init on GpSimd (no dependencies; runs at t=0)
            nc.gpsimd.memset(Mb[:, :], NEG)
            nc.gpsimd.memset(Pb[:, :], NEG)
            # preload the ScalarE activation table before any data arrives
            nc.scalar.activation(out=warm[:, :], in_=warm[:, :], func=Act.Square)

            # ---- loads (inputs are pre-cast to f16 on the host, halving
            # DMA bytes); descriptor generation split across two queues ----
            yt_h = [io_pool.tile([128, N * D], f16, tag=f"yth{c}", name=f"yth{c}")
                    for c in range(NT)]
            yp_h = [io_pool.tile([128, N * D], f16, tag=f"yph{c}", name=f"yph{c}")
                    for c in range(NT)]
            nc.sync.dma_start(out=yt_h[0][:, :], in_=yt_d[0:128, :])
            nc.scalar.dma_start(out=yp_h[0][:, :], in_=yp_d[0:128, :])
            nc.sync.dma_start(out=yt_h[1][:, :], in_=yt_d[128:256, :])
            nc.scalar.dma_start(out=yp_h[1][:, :], in_=yp_d[128:256, :])

            # ---- G[c][i*8+j] = dot(yt_i, yp_j): per-chunk broadcast
            # multiply, then one shared fold tree over both chunks ----
            prod = res_pool.tile([128, NT * N * N * D], f16, tag="prod")
            Q = NT * N * N          # 128 dot-product segments
            pv = prod.rearrange("p (q d) -> p q d", d=D)
            half = res_pool.tile([128, Q * D // 2], f16, tag="half")
            hv = half.rearrange("p (q d) -> p q d", d=D // 2)
            for c in range(NT):
                yt_b = yt_h[c].rearrange("p (i d) -> p i d", d=D).unsqueeze(2) \
                    .broadcast_to([128, N, N, D])
                yp_b = yp_h[c].rearrange("p (j d) -> p j d", d=D).unsqueeze(1) \
                    .broadcast_to([128, N, N, D])
                pc = prod[:, c * N * N * D:(c + 1) * N * N * D]
                nc.vector.tensor_tensor(
                    out=pc.rearrange("p (i j d) -> p i j d", j=N, d=D),
                    in0=yt_b, in1=yp_b, op=Alu.mult)
            nc.vector.tensor_tensor(
                out=hv, in0=pv[:, :, 0:D // 2], in1=pv[:, :, D // 2:D],
                op=Alu.add)
            qf_op = _register_quadfold_op() if (CUSTOM_HOP and USE_QF) else None
            eighth = res_pool.tile([128, Q * D // 8], f16, tag="eighth")
            if qf_op is not None:
                # quad-folds: d32 -> d8 -> d2, then a stock segmented
                # reduce d2 -> 1 that also converts to f32
                h4 = half.rearrange("p (k e) -> p k e", e=2)
                bi = nc.vector._custom_dve(
                    qf_op, out=eighth[:, :],
                    in0=h4[:, 0::2, :], in1=h4[:, 1::2, :])
                bi.ins.perf_max = 1
                half2 = res_pool.tile([128, Q * D // 32], f16, tag="half2")
                e4 = eighth.rearrange("p (k e) -> p k e", e=2)
                bi = nc.vector._custom_dve(
                    qf_op, out=half2[:, :],
                    in0=e4[:, 0::2, :], in1=e4[:, 1::2, :])
                bi.ins.perf_max = 1
                nc.vector.tensor_reduce(
                    out=G32[:, :],
                    in_=half2.rearrange("p (q d) -> p q d", d=2),
                    axis=mybir.AxisListType.X, op=Alu.add)
            else:
                quart = res_pool.tile([128, Q * D // 4], f16, tag="quart")
                qv = quart.rearrange("p (q d) -> p q d", d=D // 4)
                nc.vector.tensor_tensor(
                    out=qv, in0=hv[:, :, 0:D // 4], in1=hv[:, :, D // 4:D // 2],
                    op=Alu.add)
                ev = eighth.rearrange("p (q d) -> p q d", d=D // 8)
                nc.vector.tensor_tensor(
                    out=ev, in0=qv[:, :, 0:D // 8], in1=qv[:, :, D // 8:D // 4],
                    op=Alu.add)
                nc.vector.tensor_reduce(
                    out=G32[:, :], in_=ev, axis=mybir.AxisListType.X, op=Alu.add)

            # ---- stage 0: singleton states M[{2^i}] = G[i, t0] with
            # t0 = 0 (side A) / 4 (side B); one copy per i-pair covers all
            # 4 units: dst dims (side: +512, chunk: +256, pair: ci) from
            # src dims (t0: +4, chunk: +64, row: +8). ----
            # (on ScalarE: it is idle and this keeps the DVE on the DP)
            g4 = G32.rearrange("p (c i j) -> p c i j", c=NT, i=N)
            m4 = Mb.rearrange("p (u c s) -> p u c s", u=2, c=NT)
            for i in range(0, N, 2):
                ci = 2 ** i
                dst = m4[:, :, :, ci:2 * ci + 1:ci]
                src = g4[:, :, i:i + 2, 0:5:4].rearrange("p c i j -> p j c i")
                nc.scalar.copy(dst, src)
            # per-sample squared-norm totals on ScalarE, emitted after the
            # singles so they overlap the DP instead of contending with the
            # DVE multiplies for the input tiles' SBUF ports
            for c in range(NT):
                nc.scalar.activation(out=sq[:, :], in_=yt_h[c][:, :],
                                     func=Act.Square, accum_out=ntt[:, c:c + 1])
                nc.scalar.activation(out=sq[:, :], in_=yp_h[c][:, :],
                                     func=Act.Square, accum_out=npt[:, c:c + 1])

            # ---- DP stages t = 1..3 ----
            # hop (side, c, i, t): scalar column c*64 + i*8 + t + 4*side
            hop_op = _register_hop_op() if CUSTOM_HOP else None
            bufs = [Mb, Pb]
            for t in range(1, 4):
                old, new = bufs[(t + 1) % 2], bufs[t % 2]
                for i in range(N):
                    ci = 2 ** i
                    for u in range(4):
                        side, c = u >> 1, u & 1
                        base = u * 256
                        col = c * 64 + i * 8 + t + 4 * side
                        ov = old[:, base:base + 256].rearrange(
                            "p (a b s) -> p a b s", b=2, s=ci)
                        nv = new[:, base:base + 256].rearrange(
                            "p (a b s) -> p a b s", b=2, s=ci)
                        if hop_op is not None:
                            bi = nc.vector._custom_dve(
                                hop_op, out=nv[:, :, 1, :], in0=ov[:, :, 0, :],
                                in1=nv[:, :, 1, :],
                                s0=G32[:, col:col + 1])
                            bi.ins.perf_max = 1
                        else:
                            nc.vector.scalar_tensor_tensor(
                                out=nv[:, :, 1, :], in0=ov[:, :, 0, :],
                                scalar=G16[:, col:col + 1],
                                in1=nv[:, :, 1, :], op0=Alu.add, op1=Alu.max)

            # ---- combine: dmax[c] = max_S A_c[S] + B_c[~S] (fused custom
            # op: add with a MAX accumulator) ----
            comb_op = _register_combine_op()
            fin = bufs[1]                                  # after stage 3
            for c in range(NT):
                a_ap = fin[:, c * 256:(c + 1) * 256]
                b_rev = fin[:, 512 + c * 256:512 + (c + 1) * 256][:, 255::-1]
                nc.vector._custom_dve(
                    comb_op, out=Mc[:, c * 256:(c + 1) * 256],
                    in0=a_ap, in1=b_rev,
                    accum_out=dmax[:, c:c + 1])

            # loss per sample: nt + np - 2*dmax  (final /(64*8*B) on host)
            nc.vector.tensor_add(s_all[:, :], ntt[:, :], npt[:, :])
            nc.vector.scalar_tensor_tensor(
                out=loss_t[:, :], in0=dmax[:, :], scalar=-2.0,
                in1=s_all[:, :], op0=Alu.mult, op1=Alu.add)
            nc.sync.dma_start(out=out_d[:, :], in_=loss_t[:, :])
    nc.compile()
    return nc


def kernel(y_true: np.ndarray, y_pred: np.ndarray) -> np.ndarray:
    from concourse.bass_utils import run_bass_kernel_spmd

    if "nc" not in _CACHE:
        _CACHE["nc"] = _build()
    nc = _CACHE["nc"]

    yt = np.ascontiguousarray(
        np.asarray(y_true, dtype=np.float32).astype(np.float16)).reshape(B, N * D)
    yp = np.ascontiguousarray(
        np.asarray(y_pred, dtype=np.float32).astype(np.float16)).reshape(B, N * D)

    in_maps = [
        {
            "yt": np.ascontiguousarray(yt[c * B_LOC:(c + 1) * B_LOC]),
            "yp": np.ascontiguousarray(yp[c * B_LOC:(c + 1) * B_LOC]),
        }
        for c in range(N_CORES)
    ]
    res = run_bass_kernel_spmd(nc, in_maps, list(range(N_CORES)), trace=TRACE)
    _CACHE["last_results"] = res
    vals = np.concatenate([np.asarray(r["out"], dtype=np.float64).reshape(-1)
                           for r in res.results])
    loss = vals.mean() / (D * N)
    return np.float32(loss)
